# revision 1
# baseline (speedup 1.0000x reference)
"""ContextNet dynamic-conv kernel for 8 TRN2 NeuronCores.

Math: the reference computes, per sample b:
    gap[b]  = x[b].sum(T) / len[b]                  (C,)
    h[b]    = sigmoid(gap[b] @ w1.T + b1)           (2C,)
    w_dyn[b, co, ci, k] = h[b, 2*co + (ci>=C/2)] * W[co, ci, k]
        where W = w2.reshape(C, C, K)               (static across batch!)
    out[b]  = conv1d(x[b], w_dyn[b], pad=K//2)      (C, T)

So the per-sample conv weight is a batch-independent tensor W scaled by a
rank-ish per-sample factor S_b[ci, co] = h[b, 2co + (ci>=64)].  We build
S_b on-chip (tiny matmuls broadcast h across partitions), scale the
pre-transposed weights once per sample, and run the conv as 5 shifted
matmuls (full 128-deep contraction) accumulating in PSUM per 512-col tile.

Sharding: pure data parallel over batch B=32 -> 4 samples per core x 8.
"""

import numpy as np
from contextlib import ExitStack

import concourse.bass as bass
import concourse.bacc as bacc
import concourse.tile as tile
from concourse import mybir
from concourse.bass_utils import run_bass_kernel_spmd

B, C, T = 32, 128, 8192
K = 5
PAD = (K - 1) // 2
NCORES = 8
BL = B // NCORES          # samples per core
TCHUNK = 4096             # input DMA chunk (2 MiB per chunk)
NCHUNK = T // TCHUNK
TT = 512                  # conv tile width (one PSUM bank of f32)
NTILES = T // TT
OUT_GROUP = 4             # conv tiles batched per output DMA (1 MiB)

FP32 = mybir.dt.float32
BF16 = mybir.dt.bfloat16

AF = mybir.ActivationFunctionType
ALU = mybir.AluOpType
AXL = mybir.AxisListType


def build_nc():
    nc = bacc.Bacc("TRN2", target_bir_lowering=False, debug=False)

    x_d = nc.dram_tensor("x", [BL, C, T], FP32, kind="ExternalInput").ap()
    il_d = nc.dram_tensor("invlen", [1, BL], FP32, kind="ExternalInput").ap()
    w1t_d = nc.dram_tensor("w1t", [C, 2 * C], FP32, kind="ExternalInput").ap()
    b1_d = nc.dram_tensor("b1", [1, 2 * C], FP32, kind="ExternalInput").ap()
    wt_d = nc.dram_tensor("wt", [C, K * C], FP32, kind="ExternalInput").ap()
    ones_d = nc.dram_tensor("ones", [1, 64], FP32, kind="ExternalInput").ap()
    out_d = nc.dram_tensor("out", [BL, C, T], FP32, kind="ExternalOutput").ap()

    with ExitStack() as ctx:
        tc = ctx.enter_context(tile.TileContext(nc))

        const = ctx.enter_context(tc.tile_pool(name="const", bufs=1))
        xf = ctx.enter_context(tc.tile_pool(name="xf", bufs=2))
        xb = ctx.enter_context(tc.tile_pool(name="xb", bufs=2))
        wscp = ctx.enter_context(tc.tile_pool(name="wscp", bufs=2))
        outp = ctx.enter_context(tc.tile_pool(name="outp", bufs=3))
        small = ctx.enter_context(tc.tile_pool(name="small", bufs=3))
        pconv = ctx.enter_context(tc.tile_pool(name="pconv", bufs=4, space="PSUM"))
        ps = ctx.enter_context(tc.tile_pool(name="ps", bufs=2, space="PSUM"))
        ph = ctx.enter_context(tc.tile_pool(name="ph", bufs=2, space="PSUM"))

        wt_sb = const.tile([C, K * C], FP32)
        nc.sync.dma_start(wt_sb[:], wt_d[:])
        w1t_sb = const.tile([C, 2 * C], FP32)
        nc.sync.dma_start(w1t_sb[:], w1t_d[:])
        b1_sb = const.tile([1, 2 * C], FP32)
        nc.sync.dma_start(b1_sb[:], b1_d[:])
        il_sb = const.tile([1, BL], FP32)
        nc.sync.dma_start(il_sb[:], il_d[:])
        ones_sb = const.tile([1, 64], FP32)
        nc.sync.dma_start(ones_sb[:], ones_d[:])

        for b in range(BL):
            # ---- stream x[b] in, converting to bf16 (+halo) and reducing ----
            x_f = xf.tile([C, T], FP32)
            x_b = xb.tile([C, T + 2 * PAD], BF16)
            gap_parts = small.tile([C, NCHUNK], FP32, tag="gapp")
            nc.vector.memset(x_b[:, 0:PAD], 0.0)
            nc.vector.memset(x_b[:, T + PAD : T + 2 * PAD], 0.0)
            for c in range(NCHUNK):
                lo, hi = c * TCHUNK, (c + 1) * TCHUNK
                nc.sync.dma_start(x_f[:, lo:hi], x_d[b, :, lo:hi])
                nc.scalar.activation(
                    x_b[:, PAD + lo : PAD + hi],
                    x_f[:, lo:hi],
                    AF.Copy,
                    accum_out=gap_parts[:, c : c + 1],
                )
            gap_r = small.tile([C, 1], FP32, tag="gapr")
            nc.vector.tensor_reduce(
                gap_r[:], gap_parts[:], axis=AXL.X, op=ALU.add
            )

            # ---- h = sigmoid(gap @ w1.T * invlen + b1), as (1, 2C) row ----
            h_ps = ph.tile([1, 2 * C], FP32)
            nc.tensor.matmul(h_ps[:], lhsT=gap_r[:], rhs=w1t_sb[:], start=True, stop=True)
            h_pre = small.tile([1, 2 * C], FP32, tag="hpre")
            nc.vector.scalar_tensor_tensor(
                h_pre[:], h_ps[:], il_sb[0:1, b : b + 1], b1_sb[:],
                op0=ALU.mult, op1=ALU.add,
            )
            h_sb = small.tile([1, 2 * C], FP32, tag="h")
            nc.scalar.activation(h_sb[:], h_pre[:], AF.Sigmoid)

            # ---- S_b[ci, co] = h[2co + (ci>=64)] via contract-1 broadcast ----
            h3 = h_sb[:].rearrange("p (a two) -> p two a", two=2)  # (1, 2, 128)
            s_ps = ps.tile([C, C], FP32)
            nc.tensor.matmul(
                s_ps[0:64, :], lhsT=ones_sb[:], rhs=h3[:, 0, :], start=True, stop=True
            )
            nc.tensor.matmul(
                s_ps[64:128, :], lhsT=ones_sb[:], rhs=h3[:, 1, :], start=True, stop=True
            )

            # ---- scaled conv weights, bf16: wsc[:, k*C+co] = wt * S_b ----
            wsc = wscp.tile([C, K * C], BF16)
            for k in range(K):
                nc.vector.tensor_mul(
                    wsc[:, k * C : (k + 1) * C],
                    wt_sb[:, k * C : (k + 1) * C],
                    s_ps[:],
                )

            # ---- conv: 5 shifted matmuls per 512-wide tile ----
            for g in range(NTILES // OUT_GROUP):
                o_sb = outp.tile([C, OUT_GROUP * TT], FP32)
                for jj in range(OUT_GROUP):
                    j = g * OUT_GROUP + jj
                    pc = pconv.tile([C, TT], FP32)
                    for k in range(K):
                        nc.tensor.matmul(
                            pc[:],
                            lhsT=wsc[:, k * C : (k + 1) * C],
                            rhs=x_b[:, j * TT + k : j * TT + k + TT],
                            start=(k == 0),
                            stop=(k == K - 1),
                        )
                    nc.scalar.copy(o_sb[:, jj * TT : (jj + 1) * TT], pc[:])
                nc.scalar.dma_start(
                    out_d[b, :, g * OUT_GROUP * TT : (g + 1) * OUT_GROUP * TT], o_sb[:]
                )

    nc.compile()
    return nc


_NC_CACHE = None


def _get_nc():
    global _NC_CACHE
    if _NC_CACHE is None:
        _NC_CACHE = build_nc()
    return _NC_CACHE


def make_in_maps(x, input_lengths, w1, b1, w2):
    x = np.ascontiguousarray(np.asarray(x, dtype=np.float32))
    lens = np.asarray(input_lengths).astype(np.float64)
    invlen = (1.0 / lens).astype(np.float32)
    w1t = np.ascontiguousarray(np.asarray(w1, dtype=np.float32).T)      # (C, 2C)
    b1r = np.asarray(b1, dtype=np.float32).reshape(1, 2 * C)
    # wt[ci, k*C + co] = W[co, ci, k],  W = w2.reshape(C, C, K)
    wt = np.ascontiguousarray(
        np.asarray(w2, dtype=np.float32).reshape(C, C, K).transpose(1, 2, 0).reshape(C, K * C)
    )
    ones = np.ones((1, 64), dtype=np.float32)

    in_maps = []
    for i in range(NCORES):
        sl = slice(i * BL, (i + 1) * BL)
        in_maps.append(
            {
                "x": np.ascontiguousarray(x[sl]),
                "invlen": np.ascontiguousarray(invlen[sl].reshape(1, BL)),
                "w1t": w1t,
                "b1": b1r,
                "wt": wt,
                "ones": ones,
            }
        )
    return in_maps


def kernel(x, input_lengths, w1, b1, w2, _trace=False):
    nc = _get_nc()
    in_maps = make_in_maps(x, input_lengths, w1, b1, w2)
    res = run_bass_kernel_spmd(nc, in_maps, core_ids=list(range(NCORES)), trace=_trace)
    out = np.concatenate([res.results[i]["out"] for i in range(NCORES)], axis=0)
    if _trace:
        kernel.last_exec_time_ns = res.exec_time_ns
        kernel.last_results = res
    return out
